# revision 33
# baseline (speedup 1.0000x reference)
"""Multi-head attention TRN2 kernel (b=4, n=4096, e=128, h=4, d=32).

Sharding: 16 (batch, query-half) units over 8 cores; core c handles batch
c//2, query rows (c%2)*2048..+2048.  Each core computes q/k/v projections
for its batch (k,v over all 4096 keys), 4 attention heads over its 2048
query rows, and the output projection for those rows.  The host only
permutes/transposes inputs and concatenates outputs.

On-device layouts are transpose-free end to end:
  scoresT[k,q] = matmul(lhsT=kT_h, rhs=qT_h)        (K=32, head row-groups)
  expT = Exp(scoresT/sqrt(e))        ScalarE (exact) or VectorE via the
         Schraudolph bit-trick int16((s*128*log2e*SCALE) + 127*128 - C)
         reinterpreted as bf16 -- splits the exp work across two engines.
  att_h[d|sum, q] = matmul(lhsT=[v_h|1], rhs=expT_h): the 33rd lhsT column
         of ones folds the softmax denominator into the same matmul (no
         separate ones-matmul pass).  Heads pack into two [128,512] PSUM
         accumulators acc[x] (x=h%2) at row blocks 64*(h//2)..+33.
  normalize: sum rows -> DMA-packed [128,16] -> one cheap reciprocal ->
         DMA broadcast back to [32,512] row blocks -> per-head multiply.
  out[q, e] = 4 accumulated matmuls(lhsT=attnT_x row block, rhs=WpAB)
Softmax max-subtraction is skipped (logits are ~N(0, 0.25), |logit|<~3.5),
the value/proj biases are folded into one effective bias on the host.
x and all weights are pre-cast to bf16 on the host (PE runs 1 cyc/row).
"""

import os
import sys

sys.path.insert(0, "/opt/trn_rl_repo")
os.environ.setdefault("NEURON_RT_RESET_CORES", "1")

import numpy as np

E, H, D = 128, 4, 32
B, N = 4, 4096
NCORES = 8
NQ = N // 2  # per-core query rows
QB = 512  # query block
NKB = N // 128  # 32 key chunks
SCALE = float(1.0 / np.sqrt(np.float32(E)))
LOG2E = float(np.log2(np.e))
# Schraudolph constants for bf16-bit exp on VectorE (applied to RAW scores)
A16 = float(128.0 * LOG2E * SCALE)
B16 = float(127.0 * 128.0 - 5.0)
# which jj-iterations compute exp on VectorE (Bresenham split, ~110/256)
NVEC = int(__import__("os").environ.get("KNVEC", "110"))
KDBG = __import__("os").environ.get("KDBG", "")

_CACHE = {}

# head h -> (acc tile x, row block j): the jj-pair's two att matmuls get
# DISJOINT PE column halves (64j differs for hh=0/1) so they overlap.
# x = h%2;  j = h//2 for even heads, 1-h//2 for odd heads.
HXJ = [(h, h % 2, (h // 2) if h % 2 == 0 else 1 - (h // 2)) for h in range(H)]


def _vec_jj(jj):
    return (jj * NVEC) % 256 < NVEC


def _split_multi_waits(nc):
    """This neuronxcc build accepts at most ONE sync wait per instruction;
    Tile emits up to two.  Hoist extra waits onto same-engine NoOps."""
    from concourse import mybir as mb

    for fn in nc.m.functions:
        for blk in fn.blocks:
            insts = list(blk.instructions)
            if not any(
                i.sync_info and i.sync_info.on_wait and len(i.sync_info.on_wait) > 1
                for i in insts
            ):
                continue
            new = []
            for inst in insts:
                si = inst.sync_info
                if si is not None and si.on_wait and len(si.on_wait) > 1:
                    waits = list(si.on_wait)
                    for j, w in enumerate(waits[:-1]):
                        new.append(
                            mb.InstNoOp(
                                name=f"{inst.name}-wsplit{j}",
                                engine=inst.engine,
                                ins=[],
                                outs=[],
                                sync_info=mb.SyncInfo(on_wait=[w], on_update=[]),
                            )
                        )
                    inst.sync_info = mb.SyncInfo(
                        on_wait=[waits[-1]], on_update=list(si.on_update or [])
                    )
                new.append(inst)
            blk.instructions = new


def _build(split=True):
    import concourse.bass as bass
    import concourse.tile as tile
    from concourse import library_config, mybir
    from concourse.vector_clock import ScopedClock, VectorClock

    f32 = mybir.dt.float32
    bf16 = mybir.dt.bfloat16
    i16 = mybir.dt.int16

    class SplitDrainTileContext(tile.TileContext):
        """Final drain waits one-sem-per-instruction (walrus limit)."""

        def _drain_and_barrier(self, tick_clock, wait_clock):
            vc = tick_clock.global_clock
            n = len(vc)
            for p in range(n):
                t = vc[p]
                if t <= 0:
                    continue
                pvec = [0] * n
                pvec[p] = t
                nop_inst = self.nc.sync.nop()
                wait_clock.add_sem_waits(
                    nop_inst.ins, ScopedClock({None: VectorClock(pvec)})
                )
            self.nc.sync.drain()
            self.nc.all_engine_barrier()
            assert self.sems is not None
            popped = self.nc._tile_sem_poison_stack.pop()
            assert popped is self._sem_poison
            self.nc.clear_and_free_semaphores(list(self.sems.allocated().values()))
            self.nc.all_engine_barrier()

    nc = bass.Bass("TRN2", target_bir_lowering=False, debug=False, num_devices=NCORES)

    xT_kv = nc.dram_tensor("xT_kv", [E, N], bf16, kind="ExternalInput")
    xT_q = nc.dram_tensor("xT_q", [E, NQ], bf16, kind="ExternalInput")
    Wq = nc.dram_tensor("Wq", [E, E], bf16, kind="ExternalInput")
    Wk = nc.dram_tensor("Wk", [E, E], bf16, kind="ExternalInput")
    Wv = nc.dram_tensor("Wv", [E, E], bf16, kind="ExternalInput")
    WpAB = nc.dram_tensor("WpAB", [E, 2 * E], bf16, kind="ExternalInput")
    bq = nc.dram_tensor("bq", [E, 1], f32, kind="ExternalInput")
    bk = nc.dram_tensor("bk", [E, 1], f32, kind="ExternalInput")
    bp = nc.dram_tensor("bp", [1, E], f32, kind="ExternalInput")
    out = nc.dram_tensor("out", [NQ, E], f32, kind="ExternalOutput")
    # DRAM scratch for the rinv broadcast (DRAM APs allow 0-stride reads).
    # ExternalOutput rather than Internal: the bass2jax PJRT path binds it.
    uscr = nc.dram_tensor("uscr", [NQ // QB, H * QB], f32, kind="ExternalOutput")

    with SplitDrainTileContext(nc) as tc:
        import contextlib

        with contextlib.ExitStack() as ctx:
            consts = ctx.enter_context(tc.tile_pool(name="consts", bufs=1))
            data = ctx.enter_context(tc.tile_pool(name="data", bufs=1))
            expool = ctx.enter_context(tc.tile_pool(name="expool", bufs=6))
            nrm = ctx.enter_context(tc.tile_pool(name="nrm", bufs=2))
            outp = ctx.enter_context(tc.tile_pool(name="outp", bufs=2))

            # ---- x loads first (longest pole for the first matmul) ----
            xq_s = data.tile([E, NQ], bf16)
            xkv_s = data.tile([E, N], bf16)
            for j in range(0, NQ, QB):
                nc.gpsimd.dma_start(out=xq_s[:, j : j + QB], in_=xT_q[:, j : j + QB])
            for j in range(0, N, 1024):
                nc.gpsimd.dma_start(
                    out=xkv_s[:, j : j + 1024], in_=xT_kv[:, j : j + 1024]
                )

            # ---- constants ----
            wq_s = consts.tile([E, E], bf16)
            nc.gpsimd.dma_start(out=wq_s[:], in_=Wq[:])
            wk_s = consts.tile([E, E], bf16)
            nc.gpsimd.dma_start(out=wk_s[:], in_=Wk[:])
            wv_s = consts.tile([E, E], bf16)
            nc.gpsimd.dma_start(out=wv_s[:], in_=Wv[:])
            wpab_s = consts.tile([E, 2 * E], bf16)
            nc.gpsimd.dma_start(out=wpab_s[:], in_=WpAB[:])
            bq_s = consts.tile([E, 1], f32)
            nc.gpsimd.dma_start(out=bq_s[:], in_=bq[:])
            bk_s = consts.tile([E, 1], f32)
            nc.gpsimd.dma_start(out=bk_s[:], in_=bk[:])
            # proj bias broadcast across partitions: [1,E] -> [128,E]
            bp_s = consts.tile([E, E], f32)
            bp_bcast = bass.AP(
                tensor=bp.ap().tensor,
                offset=bp.ap().offset,
                ap=[[0, E], [1, E]],
            )
            nc.gpsimd.dma_start(out=bp_s[:], in_=bp_bcast)

            # ---- qkv projections (bf16 matmuls) ----
            qT = data.tile([E, NQ], bf16)  # [ (h d), q ] with q-bias added
            kT = data.tile([E, N], bf16)  # [ (h d), k ] with k-bias added
            # v with a 33rd column of ones per head: [keychunk, head, d|1]
            v1 = data.tile([E, NKB, H, D + 1], bf16)
            nc.vector.memset(v1[:, :, :, D : D + 1], 1.0)

            pssc = ctx.enter_context(tc.tile_pool(name="pssc", bufs=3, space="PSUM"))
            psacc = ctx.enter_context(tc.tile_pool(name="psacc", bufs=2, space="PSUM"))

            def emit_qT():
                for j in range(0, NQ, QB):
                    ps = pssc.tile([E, QB], f32, tag="scps", name=f"qps{j}")
                    nc.tensor.matmul(
                        ps[:], wq_s[:], xq_s[:, j : j + QB], start=True, stop=True
                    )
                    nc.vector.tensor_scalar_add(qT[:, j : j + QB], ps[:], bq_s[:])

            def emit_kT_chunk(c):
                j = c * QB
                ps = pssc.tile([E, QB], f32, tag="scps", name=f"kps{j}")
                nc.tensor.matmul(
                    ps[:], wk_s[:], xkv_s[:, j : j + QB], start=True, stop=True
                )
                nc.vector.tensor_scalar_add(kT[:, j : j + QB], ps[:], bk_s[:])

            def emit_v_chunk(m):
                ps = pssc.tile([E, E], f32, tag="scps", name=f"vps{m}")
                nc.tensor.matmul(
                    ps[:],
                    xkv_s[:, 128 * m : 128 * m + 128],
                    wv_s[:],
                    start=True,
                    stop=True,
                )
                nc.vector.tensor_copy(
                    v1[:, m, :, 0:D], ps[:].rearrange("p (h d) -> p h d", h=H)
                )

            emit_qT()
            emit_kT_chunk(0)

            # ---- attention ----
            # head h = 2*p + x lives in acc tile x at rows [64p, 64p+33)
            def emit_norm(iq, accs):
                """normalize query block iq: acc rows [64p,64p+32) / row 64p+32."""
                # stage the 4 sum rows into SBUF (partition-aligned copies)
                tmp = nrm.tile([E, 2 * QB], f32, tag="ntmp", name=f"nt{iq}")
                for h, x, jb in HXJ:
                    nc.vector.tensor_copy(
                        tmp[64 * jb + D : 64 * jb + D + 1, QB * x : QB * x + QB],
                        accs[x][64 * jb + D : 64 * jb + D + 1, :],
                    )
                # pack 4x[1,512] -> [128,16] so the reciprocal is cheap
                s2 = nrm.tile([E, 16], f32, tag="ns2", name=f"ns{iq}")
                if "nopack" in KDBG:
                    nc.vector.memset(s2[:], 1.0)
                else:
                    for h, x, jb in HXJ:
                        nc.gpsimd.dma_start(
                            out=s2[32 * h : 32 * h + 32, :],
                            in_=tmp[64 * jb + D : 64 * jb + D + 1, QB * x : QB * x + QB],
                        )
                r2 = nrm.tile([E, 16], f32, tag="nr2", name=f"nr{iq}")
                nc.vector.reciprocal(r2[:], s2[:])
                # unpack to DRAM scratch; DRAM source APs may broadcast
                if "nounpack" not in KDBG:
                    nc.gpsimd.dma_start(out=uscr[iq : iq + 1, :], in_=r2[:])
                attnT = [
                    nrm.tile([E, QB], bf16, tag="nat", name=f"at{iq}_{x}")
                    for x in range(2)
                ]
                rrep = [
                    nrm.tile([E, QB], f32, tag="nrr", name=f"rr{iq}_{x}")
                    for x in range(2)
                ]
                uap = uscr.ap()
                for h, x, jb in HXJ:
                    if "nobcast" in KDBG:
                        nc.vector.memset(rrep[x][64 * jb : 64 * jb + D, :], 0.001)
                    else:
                        u_b = bass.AP(
                            tensor=uap.tensor,
                            offset=uap.offset + iq * (H * QB) + QB * h,
                            ap=[[0, D], [1, QB]],
                        )
                        nc.gpsimd.dma_start(
                            out=rrep[x][64 * jb : 64 * jb + D, :], in_=u_b
                        )
                    nc.vector.tensor_mul(
                        attnT[x][64 * jb : 64 * jb + D, :],
                        accs[x][64 * jb : 64 * jb + D, :],
                        rrep[x][64 * jb : 64 * jb + D, :],
                    )
                return attnT

            def emit_proj(iq, attnT):
                """project + store query block iq"""
                q0 = iq * QB
                ob = outp.tile([E, QB], f32, tag="ob", name=f"ob{iq}")
                bp_rep = bass.AP(
                    tensor=bp_s[:].tensor,
                    offset=bp_s[:].offset,
                    ap=[list(bp_s[:].ap[0]), [0, QB // 128], [1, E]],
                )
                ob_v = ob[:].rearrange("p (m e) -> p m e", e=E)
                if "noproj" in KDBG:
                    at_v = attnT[0][:].rearrange("p (m e) -> p m e", e=E)
                    nc.vector.tensor_add(ob_v, at_v, bp_rep)
                else:
                    # one accumulation group per PE row position (a mid-group
                    # tile_position change hangs the PE), combined on DVE
                    pps = [
                        pssc.tile([E, QB], f32, tag="scps", name=f"pp{iq}_{p}")
                        for p in range(2)
                    ]
                    for m in range(QB // 128):
                        for jb in range(2):
                            for xi, (h, x, hj) in enumerate(
                                [t for t in HXJ if t[2] == jb]
                            ):
                                nc.tensor.matmul(
                                    pps[jb][:, 128 * m : 128 * m + 128],
                                    attnT[x][
                                        64 * jb : 64 * jb + D, 128 * m : 128 * m + 128
                                    ],
                                    wpab_s[64 * jb : 64 * jb + D, E * x : E * x + E],
                                    start=(xi == 0),
                                    stop=(xi == 1),
                                    skip_group_check=True,
                                    tile_position=(64 * jb, 0),
                                )
                    pp0_v = pps[0][:].rearrange("p (m e) -> p m e", e=E)
                    nc.vector.tensor_add(ob_v, pp0_v, bp_rep)
                    nc.vector.tensor_add(ob[:], ob[:], pps[1][:])
                for m in range(QB // 128):
                    nc.gpsimd.dma_start(
                        out=out[q0 + 128 * m : q0 + 128 * m + 128, :],
                        in_=ob[:, 128 * m : 128 * m + 128],
                    )

            NSC = NQ // QB * NKB * 2  # 256 half-iterations (qb, k, pair)

            def sc_tile(jj):
                return pssc.tile([E, 2 * QB], f32, tag="scps", name=f"sc{jj}")

            def emit_sc(jj, sc):
                qb, k, p = jj // (2 * NKB), (jj % (2 * NKB)) // 2, jj % 2
                q0, k0 = qb * QB, 128 * k
                for hh in range(2):
                    h = 2 * p + hh
                    nc.tensor.matmul(
                        sc[:, QB * hh : QB * hh + QB],
                        kT[D * h : D * h + D, k0 : k0 + 128],
                        qT[D * h : D * h + D, q0 : q0 + QB],
                        start=True,
                        stop=True,
                        tile_position=(D * h, 0),
                    )

            acc = {}
            pending_proj = []
            scs = {0: sc_tile(0)}
            emit_sc(0, scs[0])
            for jj in range(NSC):
                qb, k, p = jj // (2 * NKB), (jj % (2 * NKB)) // 2, jj % 2
                if qb == 0 and p == 0:
                    # stream the kv projections under the first query block
                    if k % 4 == 0 and (k // 4 + 1) < N // QB:
                        emit_kT_chunk(k // 4 + 1)
                    emit_v_chunk(k)
                if k == 0 and p == 0:
                    acc[qb] = [
                        psacc.tile([E, QB], f32, tag="accps", name=f"acc{qb}_{x}")
                        for x in range(2)
                    ]
                if jj + 1 < NSC:
                    scs[jj + 1] = sc_tile(jj + 1)
                    emit_sc(jj + 1, scs[jj + 1])
                sc = scs.pop(jj)
                ex = expool.tile([E, 2 * QB], bf16, tag="ex", name=f"ex{jj}")
                if _vec_jj(jj):
                    nc.vector.tensor_scalar(
                        ex[:].bitcast(i16),
                        sc[:],
                        A16,
                        B16,
                        mybir.AluOpType.mult,
                        mybir.AluOpType.add,
                    )
                else:
                    nc.scalar.activation(
                        out=ex[:],
                        in_=sc[:],
                        func=mybir.ActivationFunctionType.Exp,
                        scale=SCALE,
                    )
                MO = D if "m32" in KDBG else D + 1
                for hh in range(2):
                    h = 2 * p + hh
                    _, x, jb = HXJ[h]
                    nc.tensor.matmul(
                        acc[qb][x][64 * jb : 64 * jb + MO, :],
                        v1[:, k, h, 0:MO],
                        ex[:, QB * hh : QB * hh + QB],
                        start=(k == 0),
                        stop=(k == NKB - 1),
                        tile_position=(0, 64 * jb),
                        skip_group_check=True,
                    )
                if k == NKB - 1 and p == 1:
                    pending_proj.append((qb, emit_norm(qb, acc.pop(qb))))
                if k == 6 and p == 0 and pending_proj:
                    piq, pattnT = pending_proj.pop(0)
                    emit_proj(piq, pattnT)
            while pending_proj:
                piq, pattnT = pending_proj.pop(0)
                emit_proj(piq, pattnT)

    if split:
        _split_multi_waits(nc)
    return nc


def _prep_host(x, W_qkv, b_qkv, W_proj, b_proj):
    import ml_dtypes

    bf = ml_dtypes.bfloat16
    j = np.arange(E)
    h, d = j // D, j % D
    cq = h * (3 * D) + d * 3 + 0
    ck = cq + 1
    cv = cq + 2
    Wq = np.ascontiguousarray(W_qkv[:, cq].astype(bf))
    Wk = np.ascontiguousarray(W_qkv[:, ck].astype(bf))
    Wv = np.ascontiguousarray(W_qkv[:, cv].astype(bf))
    bq = np.ascontiguousarray(b_qkv[cq].reshape(E, 1), np.float32)
    bk = np.ascontiguousarray(b_qkv[ck].reshape(E, 1), np.float32)
    bv = b_qkv[cv].astype(np.float32)
    bp = (bv @ W_proj + b_proj).astype(np.float32).reshape(1, E)
    # WpAB[64p+d, 128x+e] = W_proj[32(2p+x)+d, e]  (head h=2p+x at rows 64p)
    WpAB = np.zeros((E, 2 * E), np.float32)
    for hh, xx, jb in HXJ:
        WpAB[64 * jb : 64 * jb + D, E * xx : E * xx + E] = W_proj[
            D * hh : D * hh + D, :
        ]
    WpAB = np.ascontiguousarray(WpAB.astype(bf))
    in_maps = []
    for c in range(NCORES):
        b, half = c // 2, c % 2
        xT_kv = np.ascontiguousarray(x[b].T.astype(bf))
        xT_q = np.ascontiguousarray(x[b, half * NQ : (half + 1) * NQ].T.astype(bf))
        in_maps.append(
            {
                "xT_kv": xT_kv,
                "xT_q": xT_q,
                "Wq": Wq,
                "Wk": Wk,
                "Wv": Wv,
                "WpAB": WpAB,
                "bq": bq,
                "bk": bk,
                "bp": bp,
            }
        )
    return in_maps


def kernel(x, W_qkv, b_qkv, W_proj, b_proj, _trace=False):
    x = np.asarray(x, np.float32)
    W_qkv = np.asarray(W_qkv, np.float32)
    b_qkv = np.asarray(b_qkv, np.float32)
    W_proj = np.asarray(W_proj, np.float32)
    b_proj = np.asarray(b_proj, np.float32)

    from concourse.bass_utils import run_bass_kernel_spmd

    if "nc" not in _CACHE:
        _CACHE["nc"] = _build()
    nc = _CACHE["nc"]

    in_maps = _prep_host(x, W_qkv, b_qkv, W_proj, b_proj)
    res = run_bass_kernel_spmd(
        nc, in_maps, core_ids=list(range(NCORES)), trace=_trace
    )
    out = np.empty((B, N, E), np.float32)
    for c in range(NCORES):
        b, half = c // 2, c % 2
        out[b, half * NQ : (half + 1) * NQ] = res.results[c]["out"]
    if _trace:
        _CACHE["last_result"] = res
    return out


# revision 36
# speedup vs baseline: 1.1279x; 1.1279x over previous
"""Multi-head attention TRN2 kernel (b=4, n=4096, e=128, h=4, d=32).

Sharding: 16 (batch, query-half) units over 8 cores; core c handles batch
c//2, query rows (c%2)*2048..+2048.  Each core computes q/k/v projections
for its batch (k,v over all 4096 keys), 4 attention heads over its 2048
query rows, and the output projection for those rows.  The host only
permutes/transposes inputs and concatenates outputs.

On-device layouts are transpose-free end to end:
  scoresT[k,q] = matmul(lhsT=kT_h, rhs=qT_h)        (K=32, head row-groups)
  expT = Exp(scoresT/sqrt(e))        ScalarE (exact) or VectorE via the
         Schraudolph bit-trick int16((s*128*log2e*SCALE) + 127*128 - C)
         reinterpreted as bf16 -- splits the exp work across two engines.
  att_h[d|sum, q] = matmul(lhsT=[v_h|1], rhs=expT_h): the 33rd lhsT column
         of ones folds the softmax denominator into the same matmul (no
         separate ones-matmul pass).  Heads pack into two [128,512] PSUM
         accumulators acc[x] (x=h%2) at row blocks 64*(h//2)..+33.
  normalize: sum rows -> DMA-packed [128,16] -> one cheap reciprocal ->
         DMA broadcast back to [32,512] row blocks -> per-head multiply.
  out[q, e] = 4 accumulated matmuls(lhsT=attnT_x row block, rhs=WpAB)
Softmax max-subtraction is skipped (logits are ~N(0, 0.25), |logit|<~3.5),
the value/proj biases are folded into one effective bias on the host.
x and all weights are pre-cast to bf16 on the host (PE runs 1 cyc/row).
"""

import os
import sys

sys.path.insert(0, "/opt/trn_rl_repo")
os.environ.setdefault("NEURON_RT_RESET_CORES", "1")

import numpy as np

E, H, D = 128, 4, 32
B, N = 4, 4096
NCORES = 8
NQ = N // 2  # per-core query rows
QB = 512  # query block
NKB = N // 128  # 32 key chunks
SCALE = float(1.0 / np.sqrt(np.float32(E)))
LOG2E = float(np.log2(np.e))
# Schraudolph constants for bf16-bit exp on VectorE (applied to RAW scores)
A16 = float(128.0 * LOG2E * SCALE)
B16 = float(127.0 * 128.0 - 5.0)
# which jj-iterations compute exp on VectorE (Bresenham split, ~110/256)
NVEC = int(__import__("os").environ.get("KNVEC", "110"))
KDBG = __import__("os").environ.get("KDBG", "")

_CACHE = {}

# head h -> (acc tile x, row block j).  PE concurrency comes only from
# disjoint ROW bands (rhs streams enter via array rows), so col placement
# just mirrors the pair index.
HXJ = [(h, h % 2, h // 2) for h in range(H)]


def _vec_jj(jj):
    return (jj * NVEC) % 256 < NVEC


def _split_multi_waits(nc):
    """This neuronxcc build accepts at most ONE sync wait per instruction;
    Tile emits up to two.  Hoist extra waits onto same-engine NoOps."""
    from concourse import mybir as mb

    for fn in nc.m.functions:
        for blk in fn.blocks:
            insts = list(blk.instructions)
            if not any(
                i.sync_info and i.sync_info.on_wait and len(i.sync_info.on_wait) > 1
                for i in insts
            ):
                continue
            new = []
            for inst in insts:
                si = inst.sync_info
                if si is not None and si.on_wait and len(si.on_wait) > 1:
                    waits = list(si.on_wait)
                    for j, w in enumerate(waits[:-1]):
                        new.append(
                            mb.InstNoOp(
                                name=f"{inst.name}-wsplit{j}",
                                engine=inst.engine,
                                ins=[],
                                outs=[],
                                sync_info=mb.SyncInfo(on_wait=[w], on_update=[]),
                            )
                        )
                    inst.sync_info = mb.SyncInfo(
                        on_wait=[waits[-1]], on_update=list(si.on_update or [])
                    )
                new.append(inst)
            blk.instructions = new


def _build(split=True):
    import concourse.bass as bass
    import concourse.tile as tile
    from concourse import library_config, mybir
    from concourse.vector_clock import ScopedClock, VectorClock

    f32 = mybir.dt.float32
    bf16 = mybir.dt.bfloat16
    i16 = mybir.dt.int16

    class SplitDrainTileContext(tile.TileContext):
        """Final drain waits one-sem-per-instruction (walrus limit)."""

        def _drain_and_barrier(self, tick_clock, wait_clock):
            vc = tick_clock.global_clock
            n = len(vc)
            for p in range(n):
                t = vc[p]
                if t <= 0:
                    continue
                pvec = [0] * n
                pvec[p] = t
                nop_inst = self.nc.sync.nop()
                wait_clock.add_sem_waits(
                    nop_inst.ins, ScopedClock({None: VectorClock(pvec)})
                )
            self.nc.sync.drain()
            self.nc.all_engine_barrier()
            assert self.sems is not None
            popped = self.nc._tile_sem_poison_stack.pop()
            assert popped is self._sem_poison
            self.nc.clear_and_free_semaphores(list(self.sems.allocated().values()))
            self.nc.all_engine_barrier()

    nc = bass.Bass("TRN2", target_bir_lowering=False, debug=False, num_devices=NCORES)

    xT_kv = nc.dram_tensor("xT_kv", [E, N], bf16, kind="ExternalInput")
    xT_q = nc.dram_tensor("xT_q", [E, NQ], bf16, kind="ExternalInput")
    Wq = nc.dram_tensor("Wq", [E, E], bf16, kind="ExternalInput")
    Wk = nc.dram_tensor("Wk", [E, E], bf16, kind="ExternalInput")
    Wv = nc.dram_tensor("Wv", [E, E], bf16, kind="ExternalInput")
    WpAB = nc.dram_tensor("WpAB", [E, 2 * E], bf16, kind="ExternalInput")
    bq = nc.dram_tensor("bq", [E, 1], f32, kind="ExternalInput")
    bk = nc.dram_tensor("bk", [E, 1], f32, kind="ExternalInput")
    bp = nc.dram_tensor("bp", [1, E], f32, kind="ExternalInput")
    out = nc.dram_tensor("out", [NQ, E], f32, kind="ExternalOutput")
    # DRAM scratch for the rinv broadcast (DRAM APs allow 0-stride reads).
    # ExternalOutput rather than Internal: the bass2jax PJRT path binds it.
    uscr = nc.dram_tensor("uscr", [NQ // QB, H * QB], f32, kind="ExternalOutput")

    with SplitDrainTileContext(nc) as tc:
        import contextlib

        with contextlib.ExitStack() as ctx:
            consts = ctx.enter_context(tc.tile_pool(name="consts", bufs=1))
            data = ctx.enter_context(tc.tile_pool(name="data", bufs=1))
            expool = ctx.enter_context(tc.tile_pool(name="expool", bufs=6))
            nrm = ctx.enter_context(tc.tile_pool(name="nrm", bufs=2))
            outp = ctx.enter_context(tc.tile_pool(name="outp", bufs=2))

            # ---- x loads first (longest pole for the first matmul) ----
            xq_s = data.tile([E, NQ], bf16)
            xkv_s = data.tile([E, N], bf16)
            for j in range(0, NQ, QB):
                nc.gpsimd.dma_start(out=xq_s[:, j : j + QB], in_=xT_q[:, j : j + QB])
            for j in range(0, N, 1024):
                nc.gpsimd.dma_start(
                    out=xkv_s[:, j : j + 1024], in_=xT_kv[:, j : j + 1024]
                )

            # ---- constants ----
            wq_s = consts.tile([E, E], bf16)
            nc.gpsimd.dma_start(out=wq_s[:], in_=Wq[:])
            wk_s = consts.tile([E, E], bf16)
            nc.gpsimd.dma_start(out=wk_s[:], in_=Wk[:])
            wv_s = consts.tile([E, E], bf16)
            nc.gpsimd.dma_start(out=wv_s[:], in_=Wv[:])
            wpab_s = consts.tile([E, 2 * E], bf16)
            nc.gpsimd.dma_start(out=wpab_s[:], in_=WpAB[:])
            bq_s = consts.tile([E, 1], f32)
            nc.gpsimd.dma_start(out=bq_s[:], in_=bq[:])
            bk_s = consts.tile([E, 1], f32)
            nc.gpsimd.dma_start(out=bk_s[:], in_=bk[:])
            # proj bias broadcast across partitions: [1,E] -> [128,E]
            bp_s = consts.tile([E, E], f32)
            bp_bcast = bass.AP(
                tensor=bp.ap().tensor,
                offset=bp.ap().offset,
                ap=[[0, E], [1, E]],
            )
            nc.gpsimd.dma_start(out=bp_s[:], in_=bp_bcast)

            # ---- qkv projections (bf16 matmuls) ----
            qT = data.tile([E, NQ], bf16)  # [ (h d), q ] with q-bias added
            kT = data.tile([E, N], bf16)  # [ (h d), k ] with k-bias added
            # v with a 33rd column of ones per head: [keychunk, head, d|1]
            v1 = data.tile([E, NKB, H, D + 1], bf16)
            nc.vector.memset(v1[:, :, :, D : D + 1], 1.0)

            pssc = ctx.enter_context(tc.tile_pool(name="pssc", bufs=3, space="PSUM"))
            psacc = ctx.enter_context(tc.tile_pool(name="psacc", bufs=2, space="PSUM"))

            def emit_qT():
                for j in range(0, NQ, QB):
                    ps = pssc.tile([E, QB], f32, tag="scps", name=f"qps{j}")
                    nc.tensor.matmul(
                        ps[:], wq_s[:], xq_s[:, j : j + QB], start=True, stop=True
                    )
                    nc.vector.tensor_scalar_add(qT[:, j : j + QB], ps[:], bq_s[:])

            def emit_kT_chunk(c):
                j = c * QB
                ps = pssc.tile([E, QB], f32, tag="scps", name=f"kps{j}")
                nc.tensor.matmul(
                    ps[:], wk_s[:], xkv_s[:, j : j + QB], start=True, stop=True
                )
                nc.vector.tensor_scalar_add(kT[:, j : j + QB], ps[:], bk_s[:])

            def emit_v_chunk(m):
                ps = pssc.tile([E, E], f32, tag="scps", name=f"vps{m}")
                nc.tensor.matmul(
                    ps[:],
                    xkv_s[:, 128 * m : 128 * m + 128],
                    wv_s[:],
                    start=True,
                    stop=True,
                )
                nc.vector.tensor_copy(
                    v1[:, m, :, 0:D], ps[:].rearrange("p (h d) -> p h d", h=H)
                )

            emit_qT()
            emit_kT_chunk(0)

            # ---- attention ----
            # head h = 2*p + x lives in acc tile x at rows [64p, 64p+33)
            def emit_norm(iq, accs):
                """normalize query block iq: acc rows [64p,64p+32) / row 64p+32."""
                # stage the 4 sum rows into SBUF (partition-aligned copies)
                tmp = nrm.tile([E, 2 * QB], f32, tag="ntmp", name=f"nt{iq}")
                for h, x, jb in HXJ:
                    nc.vector.tensor_copy(
                        tmp[64 * jb + D : 64 * jb + D + 1, QB * x : QB * x + QB],
                        accs[x][64 * jb + D : 64 * jb + D + 1, :],
                    )
                # pack 4x[1,512] -> [128,16] so the reciprocal is cheap
                s2 = nrm.tile([E, 16], f32, tag="ns2", name=f"ns{iq}")
                if "nopack" in KDBG:
                    nc.vector.memset(s2[:], 1.0)
                else:
                    for h, x, jb in HXJ:
                        nc.gpsimd.dma_start(
                            out=s2[32 * h : 32 * h + 32, :],
                            in_=tmp[64 * jb + D : 64 * jb + D + 1, QB * x : QB * x + QB],
                        )
                r2 = nrm.tile([E, 16], f32, tag="nr2", name=f"nr{iq}")
                nc.vector.reciprocal(r2[:], s2[:])
                # unpack to DRAM scratch; DRAM source APs may broadcast
                if "nounpack" not in KDBG:
                    nc.gpsimd.dma_start(out=uscr[iq : iq + 1, :], in_=r2[:])
                attnT = [
                    nrm.tile([E, QB], bf16, tag="nat", name=f"at{iq}_{x}")
                    for x in range(2)
                ]
                rrep = [
                    nrm.tile([E, QB], f32, tag="nrr", name=f"rr{iq}_{x}")
                    for x in range(2)
                ]
                uap = uscr.ap()
                for h, x, jb in HXJ:
                    if "nobcast" in KDBG:
                        nc.vector.memset(rrep[x][64 * jb : 64 * jb + D, :], 0.001)
                    else:
                        u_b = bass.AP(
                            tensor=uap.tensor,
                            offset=uap.offset + iq * (H * QB) + QB * h,
                            ap=[[0, D], [1, QB]],
                        )
                        nc.gpsimd.dma_start(
                            out=rrep[x][64 * jb : 64 * jb + D, :], in_=u_b
                        )
                    nc.vector.tensor_mul(
                        attnT[x][64 * jb : 64 * jb + D, :],
                        accs[x][64 * jb : 64 * jb + D, :],
                        rrep[x][64 * jb : 64 * jb + D, :],
                    )
                return attnT

            def emit_proj(iq, attnT):
                """project + store query block iq"""
                q0 = iq * QB
                ob = outp.tile([E, QB], f32, tag="ob", name=f"ob{iq}")
                bp_rep = bass.AP(
                    tensor=bp_s[:].tensor,
                    offset=bp_s[:].offset,
                    ap=[list(bp_s[:].ap[0]), [0, QB // 128], [1, E]],
                )
                ob_v = ob[:].rearrange("p (m e) -> p m e", e=E)
                if "noproj" in KDBG:
                    at_v = attnT[0][:].rearrange("p (m e) -> p m e", e=E)
                    nc.vector.tensor_add(ob_v, at_v, bp_rep)
                else:
                    # one accumulation group per PE row position (a mid-group
                    # tile_position change hangs the PE), combined on DVE
                    pps = [
                        pssc.tile([E, QB], f32, tag="scps", name=f"pp{iq}_{p}")
                        for p in range(2)
                    ]
                    for m in range(QB // 128):
                        for jb in range(2):
                            for xi, (h, x, hj) in enumerate(
                                [t for t in HXJ if t[2] == jb]
                            ):
                                nc.tensor.matmul(
                                    pps[jb][:, 128 * m : 128 * m + 128],
                                    attnT[x][
                                        64 * jb : 64 * jb + D, 128 * m : 128 * m + 128
                                    ],
                                    wpab_s[64 * jb : 64 * jb + D, E * x : E * x + E],
                                    start=(xi == 0),
                                    stop=(xi == 1),
                                    skip_group_check=True,
                                    tile_position=(64 * jb, 0),
                                )
                    pp0_v = pps[0][:].rearrange("p (m e) -> p m e", e=E)
                    nc.vector.tensor_add(ob_v, pp0_v, bp_rep)
                    nc.vector.tensor_add(ob[:], ob[:], pps[1][:])
                for m in range(QB // 128):
                    nc.gpsimd.dma_start(
                        out=out[q0 + 128 * m : q0 + 128 * m + 128, :],
                        in_=ob[:, 128 * m : 128 * m + 128],
                    )

            NSC = NQ // QB * NKB * 2  # 256 half-iterations (qb, k, pair)

            def sc_tile(jj):
                return pssc.tile([E, 2 * QB], f32, tag="scps", name=f"sc{jj}")

            def emit_sc(jj, sc):
                qb, k, p = jj // (2 * NKB), (jj % (2 * NKB)) // 2, jj % 2
                q0, k0 = qb * QB, 128 * k
                for hh in range(2):
                    h = 2 * p + hh
                    nc.tensor.matmul(
                        sc[:, QB * hh : QB * hh + QB],
                        kT[D * h : D * h + D, k0 : k0 + 128],
                        qT[D * h : D * h + D, q0 : q0 + QB],
                        start=True,
                        stop=True,
                        tile_position=(D * h, 0),
                    )

            MO = D if "m32" in KDBG else D + 1
            DEFER = int(os.environ.get("KDEFER", "2"))
            acc = {}
            exs = {}
            pending_normb = []
            pending_proj = []

            def emit_att(jj):
                # deferred by DEFER iterations: the ex tile is long since
                # ready, so the PE never stalls waiting on an in-flight exp
                # (continuous execution lets the p-state ramp to full clock)
                qb, k, p = jj // (2 * NKB), (jj % (2 * NKB)) // 2, jj % 2
                ex = exs.pop(jj)
                for hh in range(2):
                    h = 2 * p + hh
                    _, x, jb = HXJ[h]
                    nc.tensor.matmul(
                        acc[qb][x][64 * jb : 64 * jb + MO, :],
                        v1[:, k, h, 0:MO],
                        ex[:, QB * hh : QB * hh + QB],
                        start=(k == 0),
                        stop=(k == NKB - 1),
                        tile_position=(0, 64 * jb),
                        skip_group_check=True,
                    )
                if k == NKB - 1 and p == 1:
                    pending_normb.append((qb, acc.pop(qb)))

            scs = {0: sc_tile(0)}
            emit_sc(0, scs[0])
            for jj in range(NSC):
                qb, k, p = jj // (2 * NKB), (jj % (2 * NKB)) // 2, jj % 2
                if qb == 0 and p == 0:
                    # stream the kv projections under the first query block
                    if k % 4 == 0 and (k // 4 + 1) < N // QB:
                        emit_kT_chunk(k // 4 + 1)
                    emit_v_chunk(k)
                if k == 0 and p == 0:
                    acc[qb] = [
                        psacc.tile([E, QB], f32, tag="accps", name=f"acc{qb}_{x}")
                        for x in range(2)
                    ]
                if jj + 1 < NSC:
                    scs[jj + 1] = sc_tile(jj + 1)
                    emit_sc(jj + 1, scs[jj + 1])
                sc = scs.pop(jj)
                ex = expool.tile([E, 2 * QB], bf16, tag="ex", name=f"ex{jj}")
                exs[jj] = ex
                if _vec_jj(jj):
                    nc.vector.tensor_scalar(
                        ex[:].bitcast(i16),
                        sc[:],
                        A16,
                        B16,
                        mybir.AluOpType.mult,
                        mybir.AluOpType.add,
                    )
                else:
                    nc.scalar.activation(
                        out=ex[:],
                        in_=sc[:],
                        func=mybir.ActivationFunctionType.Exp,
                        scale=SCALE,
                    )
                if jj - DEFER >= 0:
                    emit_att(jj - DEFER)
                if k == 4 and p == 0 and pending_normb:
                    nqb, naccs = pending_normb.pop(0)
                    pending_proj.append((nqb, emit_norm(nqb, naccs)))
                if k == 8 and p == 0 and pending_proj:
                    piq, pattnT = pending_proj.pop(0)
                    emit_proj(piq, pattnT)
            for jj in range(max(0, NSC - DEFER), NSC):
                emit_att(jj)
            while pending_normb:
                nqb, naccs = pending_normb.pop(0)
                pending_proj.append((nqb, emit_norm(nqb, naccs)))
            while pending_proj:
                piq, pattnT = pending_proj.pop(0)
                emit_proj(piq, pattnT)

    if split:
        _split_multi_waits(nc)
    return nc


def _prep_host(x, W_qkv, b_qkv, W_proj, b_proj):
    import ml_dtypes

    bf = ml_dtypes.bfloat16
    j = np.arange(E)
    h, d = j // D, j % D
    cq = h * (3 * D) + d * 3 + 0
    ck = cq + 1
    cv = cq + 2
    Wq = np.ascontiguousarray(W_qkv[:, cq].astype(bf))
    Wk = np.ascontiguousarray(W_qkv[:, ck].astype(bf))
    Wv = np.ascontiguousarray(W_qkv[:, cv].astype(bf))
    bq = np.ascontiguousarray(b_qkv[cq].reshape(E, 1), np.float32)
    bk = np.ascontiguousarray(b_qkv[ck].reshape(E, 1), np.float32)
    bv = b_qkv[cv].astype(np.float32)
    bp = (bv @ W_proj + b_proj).astype(np.float32).reshape(1, E)
    # WpAB[64p+d, 128x+e] = W_proj[32(2p+x)+d, e]  (head h=2p+x at rows 64p)
    WpAB = np.zeros((E, 2 * E), np.float32)
    for hh, xx, jb in HXJ:
        WpAB[64 * jb : 64 * jb + D, E * xx : E * xx + E] = W_proj[
            D * hh : D * hh + D, :
        ]
    WpAB = np.ascontiguousarray(WpAB.astype(bf))
    in_maps = []
    for c in range(NCORES):
        b, half = c // 2, c % 2
        xT_kv = np.ascontiguousarray(x[b].T.astype(bf))
        xT_q = np.ascontiguousarray(x[b, half * NQ : (half + 1) * NQ].T.astype(bf))
        in_maps.append(
            {
                "xT_kv": xT_kv,
                "xT_q": xT_q,
                "Wq": Wq,
                "Wk": Wk,
                "Wv": Wv,
                "WpAB": WpAB,
                "bq": bq,
                "bk": bk,
                "bp": bp,
            }
        )
    return in_maps


def kernel(x, W_qkv, b_qkv, W_proj, b_proj, _trace=False):
    x = np.asarray(x, np.float32)
    W_qkv = np.asarray(W_qkv, np.float32)
    b_qkv = np.asarray(b_qkv, np.float32)
    W_proj = np.asarray(W_proj, np.float32)
    b_proj = np.asarray(b_proj, np.float32)

    from concourse.bass_utils import run_bass_kernel_spmd

    if "nc" not in _CACHE:
        _CACHE["nc"] = _build()
    nc = _CACHE["nc"]

    in_maps = _prep_host(x, W_qkv, b_qkv, W_proj, b_proj)
    res = run_bass_kernel_spmd(
        nc, in_maps, core_ids=list(range(NCORES)), trace=_trace
    )
    out = np.empty((B, N, E), np.float32)
    for c in range(NCORES):
        b, half = c // 2, c % 2
        out[b, half * NQ : (half + 1) * NQ] = res.results[c]["out"]
    if _trace:
        _CACHE["last_result"] = res
    return out


# revision 37
# speedup vs baseline: 1.1315x; 1.0031x over previous
"""Multi-head attention TRN2 kernel (b=4, n=4096, e=128, h=4, d=32).

Sharding: 16 (batch, query-half) units over 8 cores; core c handles batch
c//2, query rows (c%2)*2048..+2048.  Each core computes q/k/v projections
for its batch (k,v over all 4096 keys), 4 attention heads over its 2048
query rows, and the output projection for those rows.  The host only
permutes/transposes inputs and concatenates outputs.

On-device layouts are transpose-free end to end:
  scoresT[k,q] = matmul(lhsT=kT_h, rhs=qT_h)        (K=32, head row-groups)
  expT = Exp(scoresT/sqrt(e))        ScalarE (exact) or VectorE via the
         Schraudolph bit-trick int16((s*128*log2e*SCALE) + 127*128 - C)
         reinterpreted as bf16 -- splits the exp work across two engines.
  att_h[d|sum, q] = matmul(lhsT=[v_h|1], rhs=expT_h): the 33rd lhsT column
         of ones folds the softmax denominator into the same matmul (no
         separate ones-matmul pass).  Heads pack into two [128,512] PSUM
         accumulators acc[x] (x=h%2) at row blocks 64*(h//2)..+33.
  normalize: sum rows -> DMA-packed [128,16] -> one cheap reciprocal ->
         DMA broadcast back to [32,512] row blocks -> per-head multiply.
  out[q, e] = 4 accumulated matmuls(lhsT=attnT_x row block, rhs=WpAB)
Softmax max-subtraction is skipped (logits are ~N(0, 0.25), |logit|<~3.5),
the value/proj biases are folded into one effective bias on the host.
x and all weights are pre-cast to bf16 on the host (PE runs 1 cyc/row).
"""

import os
import sys

sys.path.insert(0, "/opt/trn_rl_repo")
os.environ.setdefault("NEURON_RT_RESET_CORES", "1")

import numpy as np

E, H, D = 128, 4, 32
B, N = 4, 4096
NCORES = 8
NQ = N // 2  # per-core query rows
QB = 512  # query block
NKB = N // 128  # 32 key chunks
SCALE = float(1.0 / np.sqrt(np.float32(E)))
LOG2E = float(np.log2(np.e))
# Schraudolph constants for bf16-bit exp on VectorE (applied to RAW scores)
A16 = float(128.0 * LOG2E * SCALE)
B16 = float(127.0 * 128.0 - 5.0)
# which jj-iterations compute exp on VectorE (Bresenham split, ~110/256)
NVEC = int(__import__("os").environ.get("KNVEC", "110"))
KDBG = __import__("os").environ.get("KDBG", "")

_CACHE = {}

# head h -> (acc tile x, row block j).  PE concurrency comes only from
# disjoint ROW bands (rhs streams enter via array rows), so col placement
# just mirrors the pair index.
HXJ = [(h, h % 2, h // 2) for h in range(H)]


def _vec_jj(jj):
    return (jj * NVEC) % 256 < NVEC


def _split_multi_waits(nc):
    """This neuronxcc build accepts at most ONE sync wait per instruction;
    Tile emits up to two.  Hoist extra waits onto same-engine NoOps."""
    from concourse import mybir as mb

    for fn in nc.m.functions:
        for blk in fn.blocks:
            insts = list(blk.instructions)
            if not any(
                i.sync_info and i.sync_info.on_wait and len(i.sync_info.on_wait) > 1
                for i in insts
            ):
                continue
            new = []
            for inst in insts:
                si = inst.sync_info
                if si is not None and si.on_wait and len(si.on_wait) > 1:
                    waits = list(si.on_wait)
                    for j, w in enumerate(waits[:-1]):
                        new.append(
                            mb.InstNoOp(
                                name=f"{inst.name}-wsplit{j}",
                                engine=inst.engine,
                                ins=[],
                                outs=[],
                                sync_info=mb.SyncInfo(on_wait=[w], on_update=[]),
                            )
                        )
                    inst.sync_info = mb.SyncInfo(
                        on_wait=[waits[-1]], on_update=list(si.on_update or [])
                    )
                new.append(inst)
            blk.instructions = new


def _build(split=True):
    import concourse.bass as bass
    import concourse.tile as tile
    from concourse import library_config, mybir
    from concourse.vector_clock import ScopedClock, VectorClock

    f32 = mybir.dt.float32
    bf16 = mybir.dt.bfloat16
    i16 = mybir.dt.int16

    class SplitDrainTileContext(tile.TileContext):
        """Final drain waits one-sem-per-instruction (walrus limit)."""

        def _drain_and_barrier(self, tick_clock, wait_clock):
            vc = tick_clock.global_clock
            n = len(vc)
            for p in range(n):
                t = vc[p]
                if t <= 0:
                    continue
                pvec = [0] * n
                pvec[p] = t
                nop_inst = self.nc.sync.nop()
                wait_clock.add_sem_waits(
                    nop_inst.ins, ScopedClock({None: VectorClock(pvec)})
                )
            self.nc.sync.drain()
            self.nc.all_engine_barrier()
            assert self.sems is not None
            popped = self.nc._tile_sem_poison_stack.pop()
            assert popped is self._sem_poison
            self.nc.clear_and_free_semaphores(list(self.sems.allocated().values()))
            self.nc.all_engine_barrier()

    nc = bass.Bass("TRN2", target_bir_lowering=False, debug=False, num_devices=NCORES)

    xT_kv = nc.dram_tensor("xT_kv", [E, N], bf16, kind="ExternalInput")
    xT_q = nc.dram_tensor("xT_q", [E, NQ], bf16, kind="ExternalInput")
    Wq = nc.dram_tensor("Wq", [E, E], bf16, kind="ExternalInput")
    Wk = nc.dram_tensor("Wk", [E, E], bf16, kind="ExternalInput")
    Wv = nc.dram_tensor("Wv", [E, E], bf16, kind="ExternalInput")
    WpAB = nc.dram_tensor("WpAB", [E, 2 * E], bf16, kind="ExternalInput")
    bq = nc.dram_tensor("bq", [E, 1], f32, kind="ExternalInput")
    bk = nc.dram_tensor("bk", [E, 1], f32, kind="ExternalInput")
    bp = nc.dram_tensor("bp", [1, E], f32, kind="ExternalInput")
    out = nc.dram_tensor("out", [NQ, E], f32, kind="ExternalOutput")
    # DRAM scratch for the rinv broadcast (DRAM APs allow 0-stride reads).
    # ExternalOutput rather than Internal: the bass2jax PJRT path binds it.
    uscr = nc.dram_tensor("uscr", [NQ // QB, H * QB], f32, kind="ExternalOutput")

    with SplitDrainTileContext(nc) as tc:
        import contextlib

        with contextlib.ExitStack() as ctx:
            consts = ctx.enter_context(tc.tile_pool(name="consts", bufs=1))
            data = ctx.enter_context(tc.tile_pool(name="data", bufs=1))
            expool = ctx.enter_context(tc.tile_pool(name="expool", bufs=6))
            nrm = ctx.enter_context(tc.tile_pool(name="nrm", bufs=2))
            outp = ctx.enter_context(tc.tile_pool(name="outp", bufs=2))

            # ---- x loads first (longest pole for the first matmul) ----
            xq_s = data.tile([E, NQ], bf16)
            xkv_s = data.tile([E, N], bf16)
            for j in range(0, NQ, QB):
                nc.gpsimd.dma_start(out=xq_s[:, j : j + QB], in_=xT_q[:, j : j + QB])
            for j in range(0, N, 1024):
                nc.gpsimd.dma_start(
                    out=xkv_s[:, j : j + 1024], in_=xT_kv[:, j : j + 1024]
                )

            # ---- constants ----
            wq_s = consts.tile([E, E], bf16)
            nc.gpsimd.dma_start(out=wq_s[:], in_=Wq[:])
            wk_s = consts.tile([E, E], bf16)
            nc.gpsimd.dma_start(out=wk_s[:], in_=Wk[:])
            wv_s = consts.tile([E, E], bf16)
            nc.gpsimd.dma_start(out=wv_s[:], in_=Wv[:])
            wpab_s = consts.tile([E, 2 * E], bf16)
            nc.gpsimd.dma_start(out=wpab_s[:], in_=WpAB[:])
            bq_s = consts.tile([E, 1], f32)
            nc.gpsimd.dma_start(out=bq_s[:], in_=bq[:])
            bk_s = consts.tile([E, 1], f32)
            nc.gpsimd.dma_start(out=bk_s[:], in_=bk[:])
            # proj bias broadcast across partitions: [1,E] -> [128,E]
            bp_s = consts.tile([E, E], f32)
            bp_bcast = bass.AP(
                tensor=bp.ap().tensor,
                offset=bp.ap().offset,
                ap=[[0, E], [1, E]],
            )
            nc.gpsimd.dma_start(out=bp_s[:], in_=bp_bcast)

            # ---- qkv projections (bf16 matmuls) ----
            qT = data.tile([E, NQ], bf16)  # [ (h d), q ] with q-bias added
            kT = data.tile([E, N], bf16)  # [ (h d), k ] with k-bias added
            # v with a 33rd column of ones per head: [keychunk, head, d|1]
            v1 = data.tile([E, NKB, H, D + 1], bf16)
            nc.vector.memset(v1[:, :, :, D : D + 1], 1.0)

            pssc = ctx.enter_context(tc.tile_pool(name="pssc", bufs=3, space="PSUM"))
            psacc = ctx.enter_context(tc.tile_pool(name="psacc", bufs=2, space="PSUM"))

            def emit_qT():
                for j in range(0, NQ, QB):
                    ps = pssc.tile([E, QB], f32, tag="scps", name=f"qps{j}")
                    nc.tensor.matmul(
                        ps[:], wq_s[:], xq_s[:, j : j + QB], start=True, stop=True
                    )
                    nc.vector.tensor_scalar_add(qT[:, j : j + QB], ps[:], bq_s[:])

            def emit_kT_chunk(c):
                j = c * QB
                ps = pssc.tile([E, QB], f32, tag="scps", name=f"kps{j}")
                nc.tensor.matmul(
                    ps[:], wk_s[:], xkv_s[:, j : j + QB], start=True, stop=True
                )
                nc.vector.tensor_scalar_add(kT[:, j : j + QB], ps[:], bk_s[:])

            def emit_v_chunk(m):
                ps = pssc.tile([E, E], f32, tag="scps", name=f"vps{m}")
                nc.tensor.matmul(
                    ps[:],
                    xkv_s[:, 128 * m : 128 * m + 128],
                    wv_s[:],
                    start=True,
                    stop=True,
                )
                nc.vector.tensor_copy(
                    v1[:, m, :, 0:D], ps[:].rearrange("p (h d) -> p h d", h=H)
                )

            emit_qT()
            emit_kT_chunk(0)

            # ---- attention ----
            # head h = 2*p + x lives in acc tile x at rows [64p, 64p+33)
            # normalize is split into 3 phases, emitted several iterations
            # apart so no vector op ever waits on an in-flight DMA (which
            # would block the in-order vector queue and starve the exp).
            def norm_copies(iq, accs):
                """phase A: stage sum rows to SBUF, pack-DMA to [128,16]."""
                tmp = nrm.tile([E, 2 * QB], f32, tag="ntmp", name=f"nt{iq}")
                for h, x, jb in HXJ:
                    nc.vector.tensor_copy(
                        tmp[64 * jb + D : 64 * jb + D + 1, QB * x : QB * x + QB],
                        accs[x][64 * jb + D : 64 * jb + D + 1, :],
                    )
                s2 = nrm.tile([E, 16], f32, tag="ns2", name=f"ns{iq}")
                for h, x, jb in HXJ:
                    nc.gpsimd.dma_start(
                        out=s2[32 * h : 32 * h + 32, :],
                        in_=tmp[64 * jb + D : 64 * jb + D + 1, QB * x : QB * x + QB],
                    )
                return (accs, s2)

            def norm_recip(iq, ctx):
                """phase B: cheap reciprocal, unpack + broadcast DMAs."""
                accs, s2 = ctx
                r2 = nrm.tile([E, 16], f32, tag="nr2", name=f"nr{iq}")
                nc.vector.reciprocal(r2[:], s2[:])
                nc.gpsimd.dma_start(out=uscr[iq : iq + 1, :], in_=r2[:])
                rrep = [
                    nrm.tile([E, QB], f32, tag="nrr", name=f"rr{iq}_{x}")
                    for x in range(2)
                ]
                uap = uscr.ap()
                for h, x, jb in HXJ:
                    u_b = bass.AP(
                        tensor=uap.tensor,
                        offset=uap.offset + iq * (H * QB) + QB * h,
                        ap=[[0, D], [1, QB]],
                    )
                    nc.gpsimd.dma_start(out=rrep[x][64 * jb : 64 * jb + D, :], in_=u_b)
                return (accs, rrep)

            def norm_mul(iq, ctx):
                """phase C: per-head normalize multiplies -> attnT."""
                accs, rrep = ctx
                attnT = [
                    nrm.tile([E, QB], bf16, tag="nat", name=f"at{iq}_{x}")
                    for x in range(2)
                ]
                for h, x, jb in HXJ:
                    nc.vector.tensor_mul(
                        attnT[x][64 * jb : 64 * jb + D, :],
                        accs[x][64 * jb : 64 * jb + D, :],
                        rrep[x][64 * jb : 64 * jb + D, :],
                    )
                return attnT

            def emit_proj(iq, attnT):
                """project + store query block iq"""
                q0 = iq * QB
                ob = outp.tile([E, QB], f32, tag="ob", name=f"ob{iq}")
                bp_rep = bass.AP(
                    tensor=bp_s[:].tensor,
                    offset=bp_s[:].offset,
                    ap=[list(bp_s[:].ap[0]), [0, QB // 128], [1, E]],
                )
                ob_v = ob[:].rearrange("p (m e) -> p m e", e=E)
                if "noproj" in KDBG:
                    at_v = attnT[0][:].rearrange("p (m e) -> p m e", e=E)
                    nc.vector.tensor_add(ob_v, at_v, bp_rep)
                else:
                    # one accumulation group per PE row position (a mid-group
                    # tile_position change hangs the PE), combined on DVE
                    pps = [
                        pssc.tile([E, QB], f32, tag="scps", name=f"pp{iq}_{p}")
                        for p in range(2)
                    ]
                    for m in range(QB // 128):
                        for jb in range(2):
                            for xi, (h, x, hj) in enumerate(
                                [t for t in HXJ if t[2] == jb]
                            ):
                                nc.tensor.matmul(
                                    pps[jb][:, 128 * m : 128 * m + 128],
                                    attnT[x][
                                        64 * jb : 64 * jb + D, 128 * m : 128 * m + 128
                                    ],
                                    wpab_s[64 * jb : 64 * jb + D, E * x : E * x + E],
                                    start=(xi == 0),
                                    stop=(xi == 1),
                                    skip_group_check=True,
                                    tile_position=(64 * jb, 0),
                                )
                    pp0_v = pps[0][:].rearrange("p (m e) -> p m e", e=E)
                    nc.vector.tensor_add(ob_v, pp0_v, bp_rep)
                    nc.vector.tensor_add(ob[:], ob[:], pps[1][:])
                for m in range(QB // 128):
                    nc.gpsimd.dma_start(
                        out=out[q0 + 128 * m : q0 + 128 * m + 128, :],
                        in_=ob[:, 128 * m : 128 * m + 128],
                    )

            NSC = NQ // QB * NKB * 2  # 256 half-iterations (qb, k, pair)

            def sc_tile(jj):
                return pssc.tile([E, 2 * QB], f32, tag="scps", name=f"sc{jj}")

            def emit_sc(jj, sc):
                qb, k, p = jj // (2 * NKB), (jj % (2 * NKB)) // 2, jj % 2
                q0, k0 = qb * QB, 128 * k
                for hh in range(2):
                    h = 2 * p + hh
                    nc.tensor.matmul(
                        sc[:, QB * hh : QB * hh + QB],
                        kT[D * h : D * h + D, k0 : k0 + 128],
                        qT[D * h : D * h + D, q0 : q0 + QB],
                        start=True,
                        stop=True,
                        tile_position=(D * h, 0),
                    )

            MO = D if "m32" in KDBG else D + 1
            DEFER = int(os.environ.get("KDEFER", "2"))
            acc = {}
            exs = {}
            pending_normb = []
            pending_mul = []
            pending_proj = []

            def emit_att(jj):
                # deferred by DEFER iterations: the ex tile is long since
                # ready, so the PE never stalls waiting on an in-flight exp
                # (continuous execution lets the p-state ramp to full clock)
                qb, k, p = jj // (2 * NKB), (jj % (2 * NKB)) // 2, jj % 2
                ex = exs.pop(jj)
                for hh in range(2):
                    h = 2 * p + hh
                    _, x, jb = HXJ[h]
                    nc.tensor.matmul(
                        acc[qb][x][64 * jb : 64 * jb + MO, :],
                        v1[:, k, h, 0:MO],
                        ex[:, QB * hh : QB * hh + QB],
                        start=(k == 0),
                        stop=(k == NKB - 1),
                        tile_position=(0, 64 * jb),
                        skip_group_check=True,
                    )
                if k == NKB - 1 and p == 1:
                    pending_normb.append((qb, norm_copies(qb, acc.pop(qb))))

            scs = {0: sc_tile(0)}
            emit_sc(0, scs[0])
            for jj in range(NSC):
                qb, k, p = jj // (2 * NKB), (jj % (2 * NKB)) // 2, jj % 2
                if qb == 0 and p == 0:
                    # stream the kv projections under the first query block
                    if k % 4 == 0 and (k // 4 + 1) < N // QB:
                        emit_kT_chunk(k // 4 + 1)
                    emit_v_chunk(k)
                if k == 0 and p == 0:
                    acc[qb] = [
                        psacc.tile([E, QB], f32, tag="accps", name=f"acc{qb}_{x}")
                        for x in range(2)
                    ]
                if jj + 1 < NSC:
                    scs[jj + 1] = sc_tile(jj + 1)
                    emit_sc(jj + 1, scs[jj + 1])
                sc = scs.pop(jj)
                ex = expool.tile([E, 2 * QB], bf16, tag="ex", name=f"ex{jj}")
                exs[jj] = ex
                if _vec_jj(jj):
                    nc.vector.tensor_scalar(
                        ex[:].bitcast(i16),
                        sc[:],
                        A16,
                        B16,
                        mybir.AluOpType.mult,
                        mybir.AluOpType.add,
                    )
                else:
                    nc.scalar.activation(
                        out=ex[:],
                        in_=sc[:],
                        func=mybir.ActivationFunctionType.Exp,
                        scale=SCALE,
                    )
                if jj - DEFER >= 0:
                    emit_att(jj - DEFER)
                if k == 4 and p == 0 and pending_normb:
                    nqb, nctx = pending_normb.pop(0)
                    pending_mul.append((nqb, norm_recip(nqb, nctx)))
                if k == 8 and p == 0 and pending_mul:
                    nqb, nctx = pending_mul.pop(0)
                    pending_proj.append((nqb, norm_mul(nqb, nctx)))
                if k == 10 and p == 0 and pending_proj:
                    piq, pattnT = pending_proj.pop(0)
                    emit_proj(piq, pattnT)
            for jj in range(max(0, NSC - DEFER), NSC):
                emit_att(jj)
            while pending_normb:
                nqb, nctx = pending_normb.pop(0)
                pending_mul.append((nqb, norm_recip(nqb, nctx)))
            while pending_mul:
                nqb, nctx = pending_mul.pop(0)
                pending_proj.append((nqb, norm_mul(nqb, nctx)))
            while pending_proj:
                piq, pattnT = pending_proj.pop(0)
                emit_proj(piq, pattnT)

    if split:
        _split_multi_waits(nc)
    return nc


def _prep_host(x, W_qkv, b_qkv, W_proj, b_proj):
    import ml_dtypes

    bf = ml_dtypes.bfloat16
    j = np.arange(E)
    h, d = j // D, j % D
    cq = h * (3 * D) + d * 3 + 0
    ck = cq + 1
    cv = cq + 2
    Wq = np.ascontiguousarray(W_qkv[:, cq].astype(bf))
    Wk = np.ascontiguousarray(W_qkv[:, ck].astype(bf))
    Wv = np.ascontiguousarray(W_qkv[:, cv].astype(bf))
    bq = np.ascontiguousarray(b_qkv[cq].reshape(E, 1), np.float32)
    bk = np.ascontiguousarray(b_qkv[ck].reshape(E, 1), np.float32)
    bv = b_qkv[cv].astype(np.float32)
    bp = (bv @ W_proj + b_proj).astype(np.float32).reshape(1, E)
    # WpAB[64p+d, 128x+e] = W_proj[32(2p+x)+d, e]  (head h=2p+x at rows 64p)
    WpAB = np.zeros((E, 2 * E), np.float32)
    for hh, xx, jb in HXJ:
        WpAB[64 * jb : 64 * jb + D, E * xx : E * xx + E] = W_proj[
            D * hh : D * hh + D, :
        ]
    WpAB = np.ascontiguousarray(WpAB.astype(bf))
    in_maps = []
    for c in range(NCORES):
        b, half = c // 2, c % 2
        xT_kv = np.ascontiguousarray(x[b].T.astype(bf))
        xT_q = np.ascontiguousarray(x[b, half * NQ : (half + 1) * NQ].T.astype(bf))
        in_maps.append(
            {
                "xT_kv": xT_kv,
                "xT_q": xT_q,
                "Wq": Wq,
                "Wk": Wk,
                "Wv": Wv,
                "WpAB": WpAB,
                "bq": bq,
                "bk": bk,
                "bp": bp,
            }
        )
    return in_maps


def kernel(x, W_qkv, b_qkv, W_proj, b_proj, _trace=False):
    x = np.asarray(x, np.float32)
    W_qkv = np.asarray(W_qkv, np.float32)
    b_qkv = np.asarray(b_qkv, np.float32)
    W_proj = np.asarray(W_proj, np.float32)
    b_proj = np.asarray(b_proj, np.float32)

    from concourse.bass_utils import run_bass_kernel_spmd

    if "nc" not in _CACHE:
        _CACHE["nc"] = _build()
    nc = _CACHE["nc"]

    in_maps = _prep_host(x, W_qkv, b_qkv, W_proj, b_proj)
    res = run_bass_kernel_spmd(
        nc, in_maps, core_ids=list(range(NCORES)), trace=_trace
    )
    out = np.empty((B, N, E), np.float32)
    for c in range(NCORES):
        b, half = c // 2, c % 2
        out[b, half * NQ : (half + 1) * NQ] = res.results[c]["out"]
    if _trace:
        _CACHE["last_result"] = res
    return out
